# revision 1
# baseline (speedup 1.0000x reference)
"""depth_to_space (DCR, block=2) on 8 NeuronCores.

out[b, 2h+i, 2w+j, c] = in[b, h, w, (2i+j)*64 + c]   for in [32,64,64,256] f32.

Sharding: batch dim B=32 split as 4 examples per core (data parallel, no
communication).

Per-core kernel: the permutation collapses to strided DRAM->DRAM DMA copies,
one per output-row parity i in {0,1}:
  - fuse (j,c) -> jc in [0,128): for fixed i the source slice
    x[:, :, :, i*128:(i+1)*128] merges (b,h,w) into a single stride dim:
    [[256, b*h*w], [1, 128]] (512B contiguous runs, 1KB stride);
  - the destination y[:, i::2, :, :] merges to [[16384, b*h], [1, 8192]]
    (output rows are fully contiguous).
No SBUF, no compute engines - pure DMA.

Engine assignment (measured, loop-diff timing on HW): issuing i=0 on the SP
HWDGE ring and i=1 via GPSIMD SWDGE runs at ~96us/core (~350 GB/s HBM R+W,
~98% of the 358 GB/s per-NC budget) because the two concurrent descriptor
streams interleave the complementary 512B halves of each 1KB input row,
restoring sequential HBM read locality. Single-ring: 115us; contiguous
D2D memcpy of the same volume: 102us. SWDGE caps a DMA at <16384
descriptors, so the i=1 copy is issued as two 8192-descriptor halves.
"""

import numpy as np

import concourse.bass as bass
import concourse.mybir as mybir
from concourse.bass_utils import run_bass_kernel_spmd

B, H, W, C = 32, 64, 64, 256
KS = 2
OC = C // (KS * KS)
N_CORES = 8
BS = B // N_CORES

_nc_cache = None


def build_nc() -> bass.Bass:
    nc = bass.Bass()
    x = nc.declare_dram_parameter("x", [BS, H, W, C], mybir.dt.float32, isOutput=False)
    y = nc.declare_dram_parameter(
        "y", [BS, H * KS, W * KS, OC], mybir.dt.float32, isOutput=True
    )

    # src[:, i, :]: [[256, BS*H*W], [1, 128]] starting at element offset i*128
    src = x.rearrange("b h w (i jc) -> (b h w) i jc", i=KS)
    # dst[:, i, :]: [[16384, BS*H], [1, 8192]] starting at element offset i*8192
    dst = y.rearrange("b (h i) w c -> (b h) i (w c)", i=KS)
    n_rows = BS * H  # 256
    n_src = BS * H * W  # 16384

    with (
        nc.Block() as block,
        nc.semaphore("dma_sem") as dma_sem,
        nc.semaphore("dma_sem2") as dma_sem2,
    ):

        @block.sync
        def _(sync: bass.BassEngine):
            sync.dma_start(out=dst[:, 0, :], in_=src[:, 0, :]).then_inc(dma_sem, 16)
            sync.wait_ge(dma_sem, 16)
            sync.wait_ge(dma_sem2, 32)

        @block.gpsimd
        def _(gpsimd: bass.BassEngine):
            for hf in range(2):
                gpsimd.dma_start(
                    out=dst[hf * (n_rows // 2) : (hf + 1) * (n_rows // 2), 1, :],
                    in_=src[hf * (n_src // 2) : (hf + 1) * (n_src // 2), 1, :],
                ).then_inc(dma_sem2, 16)
            gpsimd.wait_ge(dma_sem2, 32)
            gpsimd.wait_ge(dma_sem, 16)

    return nc


def kernel(batch: np.ndarray) -> np.ndarray:
    global _nc_cache
    if _nc_cache is None:
        _nc_cache = build_nc()
    nc = _nc_cache

    batch = np.ascontiguousarray(np.asarray(batch), dtype=np.float32)
    assert batch.shape == (B, H, W, C), batch.shape

    in_maps = [{"x": batch[k * BS : (k + 1) * BS]} for k in range(N_CORES)]
    res = run_bass_kernel_spmd(nc, in_maps, list(range(N_CORES)))
    return np.concatenate([res.results[k]["y"] for k in range(N_CORES)], axis=0)



# revision 2
# speedup vs baseline: 1.1301x; 1.1301x over previous
"""depth_to_space (DCR, block=2) on 8 NeuronCores.

out[b, 2h+i, 2w+j, c] = in[b, h, w, (2i+j)*64 + c]   for in [32,64,64,256] f32.

Sharding: batch dim B=32 split as 4 examples per core (data parallel, no
communication).

Per-core kernel: the permutation collapses to strided DRAM->DRAM DMA copies,
one per output-row parity i in {0,1}:
  - fuse (j,c) -> jc in [0,128): for fixed i the source slice
    x[:, :, :, i*128:(i+1)*128] merges (b,h,w) into a single stride dim:
    [[256, b*h*w], [1, 128]] (512B contiguous runs, 1KB stride);
  - the destination y[:, i::2, :, :] merges to [[16384, b*h], [1, 8192]]
    (output rows are fully contiguous).
No SBUF, no compute engines - pure DMA.

Engine assignment (measured via loop-in-NEFF wall-diff timing on HW): the
i=0 stream runs on the SP HWDGE ring and the i=1 stream on the Activation
HWDGE ring, each split into K=16 chunks with cross-engine semaphore waits
(lookahead D=2 chunks) so the two descriptor streams stay address-aligned.
The two streams read complementary 512B halves of each 1KB input line; when
aligned, the combined HBM read stream is dense and the copy runs at
~103 us/core (~326 GB/s HBM R+W) - the same rate as a contiguous D2D memcpy
of equal volume (~103 us), i.e. at the practical HBM roofline. Unaligned
dual-ring (~128 us), the old HWDGE+SWDGE pairing (~114 us), and a single
serialized ring (~140 us) are all slower; descriptor size (512B vs 64KB)
measures as free when the access pattern is dense.
"""

import numpy as np

import concourse.bass as bass
import concourse.mybir as mybir
from concourse.bass_utils import run_bass_kernel_spmd

B, H, W, C = 32, 64, 64, 256
KS = 2
OC = C // (KS * KS)
N_CORES = 8
BS = B // N_CORES

K_CHUNKS = 16
LOOKAHEAD = 2

_nc_cache = None


def build_nc() -> bass.Bass:
    nc = bass.Bass()
    x = nc.declare_dram_parameter("x", [BS, H, W, C], mybir.dt.float32, isOutput=False)
    y = nc.declare_dram_parameter(
        "y", [BS, H * KS, W * KS, OC], mybir.dt.float32, isOutput=True
    )

    # src[:, i, :]: [[256, BS*H*W], [1, 128]] starting at element offset i*128
    src = x.rearrange("b h w (i jc) -> (b h w) i jc", i=KS)
    # dst[:, i, :]: [[16384, BS*H], [1, 8192]] starting at element offset i*8192
    dst = y.rearrange("b (h i) w c -> (b h) i (w c)", i=KS)
    n_rows = BS * H  # 256
    n_src = BS * H * W  # 16384

    K, D = K_CHUNKS, LOOKAHEAD
    cs, cr = n_src // K, n_rows // K

    with nc.Block() as block:
        sp_sems = [nc.alloc_semaphore(f"sp_c{c}") for c in range(K)]
        act_sems = [nc.alloc_semaphore(f"act_c{c}") for c in range(K)]

        def stream(eng, i, own_sems, other_sems):
            for s in own_sems:
                eng.sem_clear(s)
            for c in range(K):
                if c - D >= 0:
                    eng.wait_ge(other_sems[c - D], 16)
                eng.dma_start(
                    out=dst[c * cr : (c + 1) * cr, i, :],
                    in_=src[c * cs : (c + 1) * cs, i, :],
                ).then_inc(own_sems[c], 16)
            for s in own_sems + other_sems:
                eng.wait_ge(s, 16)

        @block.sync
        def _(sync):
            stream(sync, 0, sp_sems, act_sems)

        @block.scalar
        def _(act):
            stream(act, 1, act_sems, sp_sems)

    return nc


def kernel(batch: np.ndarray) -> np.ndarray:
    global _nc_cache
    if _nc_cache is None:
        _nc_cache = build_nc()
    nc = _nc_cache

    batch = np.ascontiguousarray(np.asarray(batch), dtype=np.float32)
    assert batch.shape == (B, H, W, C), batch.shape

    in_maps = [{"x": batch[k * BS : (k + 1) * BS]} for k in range(N_CORES)]
    res = run_bass_kernel_spmd(nc, in_maps, list(range(N_CORES)))
    return np.concatenate([res.results[k]["y"] for k in range(N_CORES)], axis=0)
